# revision 3
# baseline (speedup 1.0000x reference)
"""MoE block (B=2,S=2048,D=1024,E=8,H=4096,K=2) on 8 Trainium2 NeuronCores.

Expert-parallel: core e holds expert e's weights (w1[e], w2[e]); x and the
router are replicated. Each core computes, fully on device:
  logits = x @ router_w (+router_b)          [fp32 PE matmuls -- routing
           decisions need fp32: min top2/top3 logit gap in this data is 1.3e-4]
  top-2 via sorted per-token max (DVE MAX8)
  gate_e = [l_e >= v1] * exp(l_e - v0) / (1 + exp(v1 - v0))
           (== renormalized top-2 softmax weight of expert e, 0 if unrouted)
  y_e = gate_e * (gelu_tanh(x @ w1[e] + b1[e]) @ w2[e] + b2[e])   [bf16 PE]
The host sums the 8 per-core outputs (the "reduce only the combined outputs"
from the sharding hint).

Host-side work is data movement only: transposing x (layout prep for the
contraction), permuting router_w columns so each core's own expert is
column 0, replicating small biases, and the final 8-way add.
"""
import sys

sys.path.insert(0, "/opt/trn_rl_repo")
from contextlib import ExitStack

import numpy as np

import concourse.bass as bass
import concourse.tile as tile
from concourse import bacc, mybir
from concourse.bass_utils import run_bass_kernel_spmd

F32 = mybir.dt.float32
BF16 = mybir.dt.bfloat16
AF = mybir.ActivationFunctionType

B, S, D, E, H, K = 2, 2048, 1024, 8, 4096, 2
N = B * S


def _build(T=256, has_rb=False, has_b2=False):
    DC, HC, TC, SUB = D // 128, H // 128, N // T, T // 128
    ND, NDH = 512, D // 512

    nc = bacc.Bacc("TRN2", target_bir_lowering=False, debug=False,
                   num_devices=8)
    xt_d = nc.dram_tensor("xt", [D, N], F32, kind="ExternalInput").ap()
    rw_d = nc.dram_tensor("rw", [D, E], F32, kind="ExternalInput").ap()
    w1_d = nc.dram_tensor("w1", [D, H], F32, kind="ExternalInput").ap()
    b1_d = nc.dram_tensor("b1", [H], F32, kind="ExternalInput").ap()
    w2_d = nc.dram_tensor("w2", [H, D], F32, kind="ExternalInput").ap()
    y_d = nc.dram_tensor("y", [N, D], F32, kind="ExternalOutput").ap()
    names = ["xt", "rw", "w1", "b1", "w2"]
    rb_d = b2_d = None
    if has_rb:
        rb_d = nc.dram_tensor("rbrep", [128, E], F32, kind="ExternalInput").ap()
        names.append("rbrep")
    if has_b2:
        b2_d = nc.dram_tensor("b2rep", [128, D], F32, kind="ExternalInput").ap()
        names.append("b2rep")

    xt3 = xt_d.rearrange("(dc p) t -> p dc t", p=128)
    rw3 = rw_d.rearrange("(dc p) e -> p dc e", p=128)
    w13 = w1_d.rearrange("(dc p) h -> p dc h", p=128)
    w23 = w2_d.rearrange("(hc p) d -> p hc d", p=128)
    b12 = b1_d.rearrange("(hc p) -> p hc", p=128)
    y3 = y_d.rearrange("(tc p) d -> p tc d", p=128)

    with tile.TileContext(nc) as tc, ExitStack() as ctx:
        pool = lambda name, bufs, **kw: ctx.enter_context(
            tc.tile_pool(name=name, bufs=bufs, **kw))
        consts = pool("consts", 1)
        stage = pool("stage", 2)
        xbp = pool("xb", 2)
        htp = pool("ht", 3)
        ysp = pool("ys", 4)
        gatep = pool("gate", 2)
        lgp = pool("lg", 2)
        ypsum = pool("ypsum", SUB * NDH, space="PSUM")
        hpsum = pool("hpsum", 2, space="PSUM")
        rpsum = pool("rpsum", 2, space="PSUM")

        w1b = consts.tile([128, DC, H], BF16)
        w2b = consts.tile([128, HC, D], BF16)
        rwf = consts.tile([128, DC, E], F32)
        b1f = consts.tile([128, HC], F32)
        nc.sync.dma_start(rwf[:], rw3[:, :, :])
        nc.sync.dma_start(b1f[:], b12[:, :])
        rbrep = b2rep = None
        if has_rb:
            rbrep = consts.tile([128, E], F32)
            nc.sync.dma_start(rbrep[:], rb_d[:, :])
        if has_b2:
            b2rep = consts.tile([128, D], F32)
            nc.sync.dma_start(b2rep[:], b2_d[:, :])

        for dc in range(DC):
            for hs in range(0, H, 2048):
                st = stage.tile([128, 2048], F32, tag="stage")
                nc.sync.dma_start(st[:], w13[:, dc, hs:hs + 2048])
                nc.any.tensor_copy(w1b[:, dc, hs:hs + 2048], st[:])
        for hc0 in range(0, HC, 2):
            st = stage.tile([128, 2, D], F32, tag="stage")
            nc.sync.dma_start(st[:], w23[:, hc0:hc0 + 2, :])
            nc.any.tensor_copy(w2b[:, hc0:hc0 + 2, :], st[:])

        for tci in range(TC):
            t0 = tci * T
            xs = stage.tile([128, DC, T], F32, tag="stage")
            nc.sync.dma_start(xs[:], xt3[:, :, t0:t0 + T])
            xb = xbp.tile([128, DC, T], BF16)
            nc.any.tensor_copy(xb[:], xs[:])

            lgs = lgp.tile([128, SUB, E], F32)
            for s in range(SUB):
                pr = rpsum.tile([128, E], F32, name=f"pr{s}")
                for dc in range(DC):
                    nc.tensor.matmul(pr[:], xs[:, dc, s * 128:(s + 1) * 128],
                                     rwf[:, dc, :],
                                     start=(dc == 0), stop=(dc == DC - 1))
                if has_rb:
                    nc.vector.tensor_add(lgs[:, s, :], pr[:], rbrep[:])
                else:
                    nc.vector.tensor_copy(lgs[:, s, :], pr[:])

            vals = lgp.tile([128, SUB, E], F32, tag="vals")
            for s in range(SUB):
                nc.vector.max(vals[:, s, :], lgs[:, s, :])
            gt = gatep.tile([128, SUB, 4], F32)
            l0, v0, v1 = lgs[:, :, 0], vals[:, :, 0], vals[:, :, 1]
            ea, eb, sel, gate = (gt[:, :, i] for i in range(4))
            nc.vector.tensor_sub(ea, l0, v0)
            nc.scalar.activation(ea, ea, AF.Exp)
            nc.vector.tensor_sub(eb, v1, v0)
            nc.scalar.activation(eb, eb, AF.Exp)
            nc.vector.tensor_scalar_add(eb, eb, 1.0)
            nc.vector.reciprocal(eb, eb)
            nc.vector.tensor_tensor(sel, l0, v1, mybir.AluOpType.is_ge)
            nc.vector.tensor_mul(gate, ea, eb)
            nc.vector.tensor_mul(gate, gate, sel)

            yps = [[ypsum.tile([128, ND], F32, tag="ypsum",
                               name=f"yps_{s}_{dh}")
                    for dh in range(NDH)] for s in range(SUB)]
            for hc in range(HC):
                ph = hpsum.tile([128, T], F32)
                for dc in range(DC):
                    nc.tensor.matmul(ph[:],
                                     w1b[:, dc, hc * 128:(hc + 1) * 128],
                                     xb[:, dc, :],
                                     start=(dc == 0), stop=(dc == DC - 1))
                ht = htp.tile([128, T], BF16)
                nc.scalar.activation(ht[:], ph[:], AF.Gelu_apprx_tanh,
                                     bias=b1f[:, hc:hc + 1])
                for s in range(SUB):
                    for dh in range(NDH):
                        nc.tensor.matmul(
                            yps[s][dh][:], ht[:, s * 128:(s + 1) * 128],
                            w2b[:, hc, dh * ND:(dh + 1) * ND],
                            start=(hc == 0), stop=(hc == HC - 1))

            for s in range(SUB):
                for dh in range(NDH):
                    ys = ysp.tile([128, ND], F32, tag="ys")
                    if has_b2:
                        nc.vector.tensor_add(ys[:], yps[s][dh][:],
                                             b2rep[:, dh * ND:(dh + 1) * ND])
                        nc.vector.tensor_scalar_mul(ys[:], ys[:], gt[:, s, 3:4])
                    else:
                        nc.vector.tensor_scalar_mul(ys[:], yps[s][dh][:],
                                                    gt[:, s, 3:4])
                    nc.sync.dma_start(
                        y3[:, tci * SUB + s, dh * ND:(dh + 1) * ND], ys[:])

    nc.compile()
    return nc, names


_CACHE = {}
_LAST_IN_MAPS = None


def kernel(x, router_w, router_b, w1, b1, w2, b2):
    global _LAST_IN_MAPS
    x = np.ascontiguousarray(np.asarray(x, np.float32))
    router_w = np.asarray(router_w, np.float32)
    router_b = np.asarray(router_b, np.float32)
    w1 = np.asarray(w1, np.float32)
    b1 = np.asarray(b1, np.float32)
    w2 = np.asarray(w2, np.float32)
    b2 = np.asarray(b2, np.float32)
    assert x.shape == (B, S, D) and w1.shape == (E, D, H)

    has_rb = bool(np.any(router_b != 0))
    has_b2 = bool(np.any(b2 != 0))
    xt = np.ascontiguousarray(x.reshape(N, D).T)
    in_maps = []
    for e in range(E):
        perm = [e] + [j for j in range(E) if j != e]
        m = {"xt": xt,
             "rw": np.ascontiguousarray(router_w[:, perm]),
             "w1": np.ascontiguousarray(w1[e]),
             "b1": np.ascontiguousarray(b1[e]),
             "w2": np.ascontiguousarray(w2[e])}
        if has_rb:
            m["rbrep"] = np.ascontiguousarray(
                np.broadcast_to(router_b[perm], (128, E)), dtype=np.float32)
        if has_b2:
            m["b2rep"] = np.ascontiguousarray(
                np.broadcast_to(b2[e], (128, D)), dtype=np.float32)
        in_maps.append(m)
    _LAST_IN_MAPS = in_maps

    key = (has_rb, has_b2)
    if key not in _CACHE:
        _CACHE[key] = _build(has_rb=has_rb, has_b2=has_b2)
    nc, _names = _CACHE[key]

    res = run_bass_kernel_spmd(nc, in_maps, core_ids=list(range(8)))
    y = np.zeros((N, D), np.float32)
    for r in res.results:
        y += r["y"]
    return y.reshape(B, S, D)


if __name__ == "__main__":
    rng = np.random.default_rng(0)
    sd, sh = 1 / np.sqrt(D), 1 / np.sqrt(H)
    demo = dict(
        x=rng.standard_normal((B, S, D)).astype(np.float32),
        router_w=rng.uniform(-sd, sd, (D, E)).astype(np.float32),
        router_b=np.zeros(E, np.float32),
        w1=rng.uniform(-sd, sd, (E, D, H)).astype(np.float32),
        b1=np.zeros((E, H), np.float32),
        w2=rng.uniform(-sh, sh, (E, H, D)).astype(np.float32),
        b2=np.zeros((E, D), np.float32),
    )
    out = kernel(**demo)
    print("kernel output", out.shape, out.dtype, float(np.abs(out).max()))
